# revision 62
# baseline (speedup 1.0000x reference)
"""GQA causal self-attention on 8 Trainium2 NeuronCores.

Problem: B=2, T=2048, C=2048, H=16 query heads, HKV=4 kv heads, HD=128.
Sharding: core (b, g) for b in {0,1}, g in {0..3} owns batch b, kv head g,
and the 4 query heads h with h % 4 == g (reference's _expand_kv maps query
head h -> kv head h % HKV).  Each core computes its heads' attention output
and a partial output projection (its 512 rows of Wp); the host sums the 4
partials per batch and adds bp.  No cross-core communication on device.

Device math per core (all matmuls fp16 operands, fp32 PSUM accumulation):
  qT[d, t] = Wq_g.T @ x_b.T      (x is fed pre-transposed from host)
  kT[d, t] = Wk_g.T @ x_b.T
  v[t, d]  = x_b @ Wv_g          (lhsT = xT tiles)
  ST[j, i] = kT_j . qT_i         (j keys on partitions, i queries free)
  A = exp(ST / sqrt(HD)) masked causally (block-skip + 0/1 mask on diagonal;
      the last 256-key pair of each i-tile only computes queries [256:512))
  den[*, i] = sum_j A[j, i]      (DVE pair-sum + one ones[128,128] matmul)
  yT[d, i] = (sum_j v[j, d] A[j, i]) / den[i]
  out[i, o] += yT.T @ Wp_g       (partial, fp16; host sums over g in fp32)
"""

import math
import os
from contextlib import ExitStack

import numpy as np

import concourse.bass as bass
import concourse.mybir as mybir
import concourse.tile as tile
from concourse import bacc, bass_utils

# The axon trace path needs antenv.axon_hooks; if the environment requests
# tracing but lacks the hook module, force tracing off instead of crashing.
if os.environ.get("BASS_TRACE"):
    try:
        import antenv.axon_hooks  # noqa: F401
    except ImportError:
        os.environ["BASS_NEVER_TRACE"] = "1"

# Problem shapes (hardcoded per contest rules).
B, T, C = 2, 2048, 2048
H, G = 16, 4
HKV = H // G          # 4 kv heads
HD = C // H           # 128 head dim
P = 128               # partitions
NH = H // HKV         # 4 local query heads per core
KT = C // P           # 16 contraction tiles for projections
TW = 512              # token tile width (matmul free dim)
NT = T // TW          # 4 token tiles
JTN = T // P          # 16 key tiles of 128
HW = TW // 2          # half tile: causal trim of the diagonal pair
SCALE = 1.0 / math.sqrt(HD)

FP = mybir.dt.float16
F32 = mybir.dt.float32

_CACHE = {}

# Set by kernel() after each run: bass_utils.BassKernelResults.
LAST_RESULT = None


def _build_bass():
    nc = bacc.Bacc("TRN2")

    xt = nc.dram_tensor("xt", [C, T], FP, kind="ExternalInput")
    wq = nc.dram_tensor("wq", [C, NH * HD], FP, kind="ExternalInput")
    wk = nc.dram_tensor("wk", [C, HD], FP, kind="ExternalInput")
    wv = nc.dram_tensor("wv", [C, HD], FP, kind="ExternalInput")
    wp = nc.dram_tensor("wp", [NH * HD, C], FP, kind="ExternalInput")
    bq = nc.dram_tensor("bq", [NH * HD], F32, kind="ExternalInput")
    bk = nc.dram_tensor("bk", [HD], F32, kind="ExternalInput")
    bv = nc.dram_tensor("bv", [HD], F32, kind="ExternalInput")
    mask = nc.dram_tensor("mask", [P, NT, TW], FP, kind="ExternalInput")
    out = nc.dram_tensor("out", [T, C], FP, kind="ExternalOutput")

    xt_r = xt.ap().rearrange("(ko p) t -> p ko t", p=P)       # [128,16,2048]
    wq_r = wq.ap().rearrange("(ko p) m -> p ko m", p=P)       # [128,16,512]
    wk_r = wk.ap().rearrange("(ko p) m -> p ko m", p=P)       # [128,16,128]
    wv_r = wv.ap().rearrange("(ko p) m -> p ko m", p=P)
    wp_r = wp.ap().rearrange("(h p) o -> p h o", p=P)         # [128,4,2048]
    bq_r = bq.ap().rearrange("(h p) -> p h", p=P)             # [128,4]
    out_r = out.ap().rearrange("(io p) o -> p io o", p=P)     # [128,16,2048]

    with tile.TileContext(nc) as tc, ExitStack() as ctx:
        consts = ctx.enter_context(tc.tile_pool(name="consts", bufs=1))
        xpool = ctx.enter_context(tc.tile_pool(name="xpool", bufs=2))
        espool = ctx.enter_context(tc.tile_pool(name="espool", bufs=10))
        mpool = ctx.enter_context(tc.tile_pool(name="mpool", bufs=2))
        opool = ctx.enter_context(tc.tile_pool(name="opool", bufs=2))
        # PSUM (8 banks): ps_s 2x[128,2,512] (4) for S pairs + q/k proj,
        # ps_y 1x[128,512] (1), ps_d 1x[128,512] (1),
        # ps_o 2x[128,512] (2) for v-proj and out-proj quarters.
        ps_s = ctx.enter_context(tc.tile_pool(name="ps_s", bufs=2, space="PSUM"))
        ps_y = ctx.enter_context(tc.tile_pool(name="ps_y", bufs=1, space="PSUM"))
        ps_d = ctx.enter_context(tc.tile_pool(name="ps_d", bufs=1, space="PSUM"))
        ps_o = ctx.enter_context(tc.tile_pool(name="ps_o", bufs=2, space="PSUM"))

        # Weights needed first, loaded in k-chunks interleaved with the first
        # x tile so the first q matmul can start as early as possible.
        # Chunks stay >=0.125MB — smaller transfers halve effective HBM
        # bandwidth (per-transfer setup cost).
        KC = 4  # k-chunks per load
        wq_sb = consts.tile([P, KT, NH * HD], FP)
        wk_sb = consts.tile([P, KT, HD], FP)
        wv_sb = consts.tile([P, KT, HD], FP)
        xtile0 = xpool.tile([P, KT, TW], FP, tag="xt", name="xtile0")
        # Single k=0 slices first: the first q matmul needs only these 0.26MB.
        nc.sync.dma_start(out=xtile0[:, 0], in_=xt_r[:, 0, 0:TW])
        nc.sync.dma_start(out=wq_sb[:, 0], in_=wq_r[:, 0])
        for c4 in range(KC):
            ks = slice(max(c4 * (KT // KC), 1), (c4 + 1) * (KT // KC))
            nc.sync.dma_start(out=xtile0[:, ks], in_=xt_r[:, ks, 0:TW])
            nc.sync.dma_start(out=wq_sb[:, ks], in_=wq_r[:, ks])
        # wk is consumed after the q k-loop, wv by the v-projection at the
        # end of the n=0 loop — both well after the early x/wq slices.
        nc.sync.dma_start(out=wk_sb, in_=wk_r)
        for c4 in range(2):
            ks = slice(c4 * 8, (c4 + 1) * 8)
            nc.sync.dma_start(out=wv_sb[:, ks], in_=wv_r[:, ks])
        bq_sb = consts.tile([P, NH], F32)
        nc.sync.dma_start(out=bq_sb, in_=bq_r)
        bk_sb = consts.tile([P, 1], F32)
        nc.sync.dma_start(out=bk_sb, in_=bk.ap().rearrange("(h p) -> p h", p=P))
        # bv broadcast across partitions (DRAM source allows partition step 0).
        bv_bc = consts.tile([P, HD], F32)
        bv_ap = bass.AP(tensor=bv.ap().tensor, offset=0, ap=[[0, P], [1, HD]])
        nc.sync.dma_start(out=bv_bc, in_=bv_ap)
        ones_sb = consts.tile([P, P], FP)
        nc.vector.memset(ones_sb, 1.0)
        dummy_sb = consts.tile([P, TW], FP)
        nc.vector.memset(dummy_sb, 0.0)

        # PE warm-up: HAM un-throttles (1.2 -> 2.4 GHz) after ~3.4us of
        # sustained matmul activity.  Run throwaway matmuls while the first
        # input DMAs land so the real matmuls start at full clock.
        for w in range(12):
            ps_warm = ps_o.tile([P, TW], F32, tag="pso", name=f"ps_warm{w}")
            nc.tensor.matmul(
                ps_warm,
                lhsT=ones_sb,
                rhs=dummy_sb,
                start=True,
                stop=True,
            )

        # Persistent activations.
        qT = consts.tile([P, NH, T], FP)       # [d, h, i]
        kT = consts.tile([P, T], FP)           # [d, j]
        v_sb = consts.tile([P, JTN, HD], FP)   # [j_in, j_tile, d]
        yT = consts.tile([P, NH, T], FP)       # [d, h, i]
        mask_sb = consts.tile([P, NT, TW], FP)

        # ---- Projections ----
        xtiles = []

        def vproj_block(n, js, xtile):
            psv = ps_o.tile([P, TW], F32, tag="pso", name=f"psv_{n}_{js}")
            for k in range(KT):
                nc.tensor.matmul(
                    psv[:, :HD],
                    lhsT=xtile[:, k, js * P:(js + 1) * P],
                    rhs=wv_sb[:, k, :],
                    start=(k == 0),
                    stop=(k == KT - 1),
                )
            jt = n * (TW // P) + js
            nc.vector.tensor_tensor(
                out=v_sb[:, jt, :],
                in0=psv[:, :HD],
                in1=bv_bc,
                op=mybir.AluOpType.add,
            )

        for n in range(NT):
            if n == 0:
                xtile = xtile0
            else:
                xtile = xpool.tile([P, KT, TW], FP, tag="xt", name=f"xtile{n}")
                for c4 in range(KC):
                    ks = slice(c4 * (KT // KC), (c4 + 1) * (KT // KC))
                    nc.sync.dma_start(
                        out=xtile[:, ks], in_=xt_r[:, ks, n * TW:(n + 1) * TW]
                    )
                if n == 2:
                    # The causal mask is first read by the it=0 attention,
                    # which overlaps the n=3 projections; load it before
                    # xtile3 so it can't arrive late.
                    nc.sync.dma_start(out=mask_sb, in_=mask.ap())
            psq01 = ps_s.tile([P, 2, TW], F32, tag="pss", name=f"psq01_{n}")
            psq23 = ps_s.tile([P, 2, TW], F32, tag="pss", name=f"psq23_{n}")

            def kproj():
                psk = ps_y.tile([P, TW], F32, tag="psy", name=f"psk_{n}")
                for k in range(KT):
                    nc.tensor.matmul(
                        psk, lhsT=wk_sb[:, k, :], rhs=xtile[:, k, :],
                        start=(k == 0), stop=(k == KT - 1),
                    )
                nc.vector.tensor_scalar(
                    out=kT[:, n * TW:(n + 1) * TW],
                    in0=psk,
                    scalar1=bk_sb,
                    scalar2=None,
                    op0=mybir.AluOpType.add,
                )

            if n == NT - 1:
                # Last tile: k-projection first, so the psy bank it occupies
                # is freed long before the it=0 attention needs it.
                kproj()
            for k in range(KT):
                st = k == 0
                sp = k == KT - 1
                for h in range(NH):
                    tgt = psq01 if h < 2 else psq23
                    nc.tensor.matmul(
                        tgt[:, h % 2, :],
                        lhsT=wq_sb[:, k, h * HD:(h + 1) * HD],
                        rhs=xtile[:, k, :],
                        start=st,
                        stop=sp,
                    )
            if n < NT - 1:
                # k-projection after the q k-loop: by then the whole xtile is
                # in SBUF, so a late wk DMA can't stall the early stream.
                kproj()
            for h in range(NH):
                tgt = psq01 if h < 2 else psq23
                nc.vector.tensor_scalar(
                    out=qT[:, h, n * TW:(n + 1) * TW],
                    in0=tgt[:, h % 2, :],
                    scalar1=bq_sb[:, h:h + 1],
                    scalar2=None,
                    op0=mybir.AluOpType.add,
                )
            # v-projection: DMA-independent by now (q/k streamed the whole
            # xtile); one single-bank accumulator per 128-token block.
            xtiles.append(xtile)
            for js in range(TW // P):
                vproj_block(n, js, xtile)

        # Weights for the later phases: load after projection work is queued.
        wp_sb = consts.tile([P, NH, C], FP)
        nc.sync.dma_start(out=wp_sb, in_=wp_r)

        # ---- Attention with interleaved output projection ----
        # Out-proj for i-tile it is emitted between the attention heads of
        # i-tile it+1 (its yT rows are complete by then), so the PE always
        # has exp-independent matmuls to chew on while ACT computes exps.
        osb_map = {}

        def out_proj_quarters(ic, qs, final=False):
            # Quarter-granular: each [128, 512] output slab rotates through a
            # single-bank PSUM tile (2 bufs), so quarter q+1's matmuls run
            # while quarter q's PSUM->SBUF copy drains.  Copies alternate
            # between ACT and DVE to balance the two engines; each quarter is
            # stored as soon as its copy lands.
            if ic not in osb_map:
                osb_map[ic] = opool.tile([P, C], FP, tag="osb", name=f"osb_{ic}")
            osb = osb_map[ic]
            for q in qs:
                pso = ps_o.tile([P, TW], F32, tag="pso", name=f"pso_{ic}_{q}")
                for h in range(NH):
                    nc.tensor.matmul(
                        pso,
                        lhsT=yT[:, h, ic * P:(ic + 1) * P],
                        rhs=wp_sb[:, h, q * TW:(q + 1) * TW],
                        start=(h == 0),
                        stop=(h == NH - 1),
                    )
                if final and q >= 2:
                    # Split the trailing copies across both engines and issue
                    # the stores from those engines' own DMA queues (sync's
                    # queue serializes at ~700ns/store) so the kernel's tail
                    # drains as fast as possible.  The very last piece is a
                    # quarter so the final transfer is tiny.
                    npiece = 2 if q == 2 else 3
                    cuts = (0, HW, TW) if q == 2 else (0, HW, HW + P, TW)
                    for piece in range(npiece):
                        psl = slice(cuts[piece], cuts[piece + 1])
                        csl = slice(q * TW + cuts[piece], q * TW + cuts[piece + 1])
                        if piece % 2 == 0:
                            nc.scalar.copy(out=osb[:, csl], in_=pso[:, psl])
                            nc.scalar.dma_start(
                                out=out_r[:, ic, csl], in_=osb[:, csl]
                            )
                        else:
                            nc.vector.tensor_copy(out=osb[:, csl], in_=pso[:, psl])
                            nc.sync.dma_start(out=out_r[:, ic, csl], in_=osb[:, csl])
                else:
                    csl = slice(q * TW, (q + 1) * TW)
                    ceng = nc.scalar.copy if q % 2 == 0 else nc.vector.tensor_copy
                    ceng(out=osb[:, csl], in_=pso)
                    nc.sync.dma_start(out=out_r[:, ic, csl], in_=osb[:, csl])

        for it in range(NT):
            isl = slice(it * TW, (it + 1) * TW)
            npair = 2 * (it + 1)
            for h in range(NH):
                psy = ps_y.tile([P, TW], F32, tag="psy", name=f"psy_{it}_{h}")
                # fp16 DVE row-accumulator; den is one ones-matmul on the
                # pair-summed accumulator.
                acc = mpool.tile([P, 2, TW], FP, tag="acc", name=f"acc_{it}_{h}")
                for pr in range(npair):
                    jt0 = 2 * pr
                    last = pr == npair - 1
                    penu = pr == npair - 2
                    # The diagonal pair only serves queries [HW:TW) of the
                    # i-tile; everything below is fully masked anyway.
                    fw = HW if last else TW
                    kdiag = jt0 - it * (TW // P)
                    pss = ps_s.tile(
                        [P, 2, TW], F32, tag="pss", name=f"pss_{it}_{h}_{pr}"
                    )
                    base = HW if last else 0
                    for u in range(2):
                        jt = jt0 + u
                        # Diagonal j-tiles only serve queries >= their own
                        # row block; skip the S columns below it.  The exp
                        # of the stale (bounded) PSUM there is zeroed by the
                        # causal mask before anything consumes it.
                        s0 = max(jt * P - it * TW - base, 0) if kdiag >= 0 else 0
                        nc.tensor.matmul(
                            pss[:, u, s0:fw],
                            lhsT=kT[:, jt * P:(jt + 1) * P],
                            rhs=qT[:, h, it * TW + base + s0:(it + 1) * TW],
                            start=True,
                            stop=True,
                        )
                    es = espool.tile([P, 2, TW], FP, tag="es")
                    if fw == TW:
                        # Both u-halves are contiguous (the pair tile spans
                        # two adjacent PSUM banks) — one flat 2D activation.
                        nc.scalar.activation(
                            out=es[:].rearrange("p a b -> p (a b)"),
                            in_=pss[:].rearrange("p a b -> p (a b)"),
                            func=mybir.ActivationFunctionType.Exp,
                            scale=SCALE,
                        )
                    else:
                        nc.scalar.activation(
                            out=es[:, :, :fw],
                            in_=pss[:, :, :fw],
                            func=mybir.ActivationFunctionType.Exp,
                            scale=SCALE,
                        )
                    if kdiag >= 0:
                        # The S trim already skipped everything above the
                        # diagonal except each j-tile's own 128-column block;
                        # only that block needs the 0/1 mask.  Shorter DVE
                        # ops also shorten the exp->AV critical chain.
                        for u in range(2):
                            s0u = max((jt0 + u) * P - it * TW - base, 0)
                            nc.vector.tensor_mul(
                                es[:, u, s0u:s0u + P],
                                es[:, u, s0u:s0u + P],
                                mask_sb[:, kdiag + u, base + s0u:base + s0u + P],
                            )
                    if pr == 0:
                        if kdiag >= 0:
                            # it=0 first pair: u=1's region below its own
                            # block was never computed — zero it in acc.
                            nc.vector.tensor_copy(out=acc[:, 0, :], in_=es[:, 0, :])
                            nc.vector.memset(acc[:, 1, 0:P], 0.0)
                            nc.vector.tensor_copy(
                                out=acc[:, 1, P:], in_=es[:, 1, P:]
                            )
                        else:
                            nc.vector.tensor_copy(
                                out=acc[:].rearrange("p a b -> p (a b)"),
                                in_=es[:].rearrange("p a b -> p (a b)"),
                            )
                    elif kdiag >= 0:
                        for u in range(2):
                            s0u = max((jt0 + u) * P - it * TW - base, 0)
                            nc.vector.tensor_tensor(
                                out=acc[:, u, base + s0u:],
                                in0=acc[:, u, base + s0u:],
                                in1=es[:, u, s0u:fw],
                                op=mybir.AluOpType.add,
                            )
                    else:
                        acc2 = acc[:].rearrange("p a b -> p (a b)")
                        nc.vector.tensor_tensor(
                            out=acc2,
                            in0=acc2,
                            in1=es[:].rearrange("p a b -> p (a b)"),
                            op=mybir.AluOpType.add,
                        )
                    # AV accumulation into psy, trimmed like S: diagonal
                    # j-tiles only contribute to queries >= their own row
                    # block (the skipped region's es is masked zeros anyway).
                    # `stop` is sim-only metadata; the ymul read is ordered
                    # by data dependency, so ragged groups are fine.
                    for u in range(2):
                        jt = jt0 + u
                        s0 = max(jt * P - it * TW - base, 0) if kdiag >= 0 else 0
                        nc.tensor.matmul(
                            psy[:, base + s0:],
                            lhsT=v_sb[:, jt, :],
                            rhs=es[:, u, s0:fw],
                            start=(jt == 0),
                            stop=(pr == npair - 1 and u == 1),
                            skip_group_check=True,
                        )
                accs = mpool.tile([P, TW], FP, tag="accs", name=f"accs_{it}_{h}")
                nc.vector.tensor_tensor(
                    out=accs, in0=acc[:, 0, :], in1=acc[:, 1, :],
                    op=mybir.AluOpType.add,
                )
                psd = ps_d.tile([P, TW], F32, tag="psd", name=f"psd_{it}_{h}")
                nc.tensor.matmul(psd, lhsT=ones_sb, rhs=accs, start=True, stop=True)
                rb = mpool.tile([P, TW], F32, tag="rb")
                if it == NT - 1 and h == NH - 1:
                    # The tail chunks gate on this head's yT; pipeline the
                    # recip+ymul in halves so the first tail chunks unblock
                    # before the full row range is normalized.
                    for qq in range(2):
                        sl = slice(qq * HW, (qq + 1) * HW)
                        gsl = slice(it * TW + qq * HW, it * TW + (qq + 1) * HW)
                        nc.vector.reciprocal_approx_fast(
                            out=rb[:, sl], in_=psd[:, sl]
                        )
                        nc.vector.tensor_mul(yT[:, h, gsl], psy[:, sl], rb[:, sl])
                else:
                    nc.vector.reciprocal_approx_fast(out=rb, in_=psd)
                    nc.vector.tensor_mul(yT[:, h, isl], psy, rb)
                if it > 0:
                    out_proj_quarters((it - 1) * (TW // P) + h, (0, 1, 2, 3))
        # Tail chunks for the last i-tile's rows.
        for h in range(NH):
            out_proj_quarters(
                (NT - 1) * (TW // P) + h, (0, 1, 2, 3), final=(h == NH - 1)
            )

    nc.compile()
    return nc


def _causal_mask_tiles():
    j = np.arange(P)[:, None, None]
    k = np.arange(NT)[None, :, None]
    i = np.arange(TW)[None, None, :]
    return (j + k * P <= i).astype(np.float16)


def kernel(x, Wkv, bkv, Wq, bq, Wp, bp):
    global LAST_RESULT
    x = np.asarray(x, np.float32)
    Wkv = np.asarray(Wkv, np.float32)
    bkv = np.asarray(bkv, np.float32)
    Wq = np.asarray(Wq, np.float32)
    bq = np.asarray(bq, np.float32)
    Wp = np.asarray(Wp, np.float32)
    bp = np.asarray(bp, np.float32)

    if "nc" not in _CACHE:
        _CACHE["nc"] = _build_bass()
    nc = _CACHE["nc"]

    mask = _causal_mask_tiles()
    CG = C // G  # 512 columns per kv head in the k/v halves of Wkv

    in_maps = []
    for b in range(B):
        xt = x[b].T.astype(np.float16)
        for g in range(HKV):
            heads = [g + HKV * u for u in range(NH)]  # h % HKV == g
            wq_g = np.concatenate(
                [Wq[:, h * HD:(h + 1) * HD] for h in heads], axis=1
            ).astype(np.float16)
            bq_g = np.concatenate([bq[h * HD:(h + 1) * HD] for h in heads])
            wp_g = np.ascontiguousarray(
                np.concatenate([Wp[h * HD:(h + 1) * HD, :] for h in heads], axis=0)
            ).astype(np.float16)
            wk_g = np.ascontiguousarray(Wkv[:, g * HD:(g + 1) * HD]).astype(np.float16)
            wv_g = np.ascontiguousarray(
                Wkv[:, CG + g * HD:CG + (g + 1) * HD]
            ).astype(np.float16)
            bk_g = np.ascontiguousarray(bkv[g * HD:(g + 1) * HD])
            bv_g = np.ascontiguousarray(bkv[CG + g * HD:CG + (g + 1) * HD])
            in_maps.append(
                {
                    "xt": xt,
                    "wq": wq_g,
                    "wk": wk_g,
                    "wv": wv_g,
                    "wp": wp_g,
                    "bq": np.ascontiguousarray(bq_g, np.float32),
                    "bk": np.ascontiguousarray(bk_g, np.float32),
                    "bv": np.ascontiguousarray(bv_g, np.float32),
                    "mask": mask,
                }
            )

    res = bass_utils.run_bass_kernel_spmd(nc, in_maps, core_ids=list(range(B * HKV)))
    LAST_RESULT = res

    out = np.zeros((B, T, C), np.float32)
    for b in range(B):
        acc = np.zeros((T, C), np.float32)
        for g in range(HKV):
            acc += res.results[b * HKV + g]["out"]
        out[b] = acc + bp[None, :]
    return out


# revision 63
# speedup vs baseline: 1.0141x; 1.0141x over previous
"""GQA causal self-attention on 8 Trainium2 NeuronCores.

Problem: B=2, T=2048, C=2048, H=16 query heads, HKV=4 kv heads, HD=128.
Sharding: core (b, g) for b in {0,1}, g in {0..3} owns batch b, kv head g,
and the 4 query heads h with h % 4 == g (reference's _expand_kv maps query
head h -> kv head h % HKV).  Each core computes its heads' attention output
and a partial output projection (its 512 rows of Wp); the host sums the 4
partials per batch and adds bp.  No cross-core communication on device.

Device math per core (all matmuls fp16 operands, fp32 PSUM accumulation):
  qT[d, t] = Wq_g.T @ x_b.T      (x is fed pre-transposed from host)
  kT[d, t] = Wk_g.T @ x_b.T
  v[t, d]  = x_b @ Wv_g          (lhsT = xT tiles)
  ST[j, i] = kT_j . qT_i         (j keys on partitions, i queries free)
  A = exp(ST / sqrt(HD)) masked causally (block-skip + 0/1 mask on diagonal;
      the last 256-key pair of each i-tile only computes queries [256:512))
  den[*, i] = sum_j A[j, i]      (DVE pair-sum + one ones[128,128] matmul)
  yT[d, i] = (sum_j v[j, d] A[j, i]) / den[i]
  out[i, o] += yT.T @ Wp_g       (partial, fp16; host sums over g in fp32)
"""

import math
import os
from contextlib import ExitStack

import numpy as np

import concourse.bass as bass
import concourse.mybir as mybir
import concourse.tile as tile
from concourse import bacc, bass_utils

# The axon trace path needs antenv.axon_hooks; if the environment requests
# tracing but lacks the hook module, force tracing off instead of crashing.
if os.environ.get("BASS_TRACE"):
    try:
        import antenv.axon_hooks  # noqa: F401
    except ImportError:
        os.environ["BASS_NEVER_TRACE"] = "1"

# Problem shapes (hardcoded per contest rules).
B, T, C = 2, 2048, 2048
H, G = 16, 4
HKV = H // G          # 4 kv heads
HD = C // H           # 128 head dim
P = 128               # partitions
NH = H // HKV         # 4 local query heads per core
KT = C // P           # 16 contraction tiles for projections
TW = 512              # token tile width (matmul free dim)
NT = T // TW          # 4 token tiles
JTN = T // P          # 16 key tiles of 128
HW = TW // 2          # half tile: causal trim of the diagonal pair
SCALE = 1.0 / math.sqrt(HD)

FP = mybir.dt.float16
F32 = mybir.dt.float32

_CACHE = {}

# Set by kernel() after each run: bass_utils.BassKernelResults.
LAST_RESULT = None


def _build_bass():
    nc = bacc.Bacc("TRN2")

    xt = nc.dram_tensor("xt", [C, T], FP, kind="ExternalInput")
    wq = nc.dram_tensor("wq", [C, NH * HD], FP, kind="ExternalInput")
    wk = nc.dram_tensor("wk", [C, HD], FP, kind="ExternalInput")
    wv = nc.dram_tensor("wv", [C, HD], FP, kind="ExternalInput")
    wp = nc.dram_tensor("wp", [NH * HD, C], FP, kind="ExternalInput")
    bq = nc.dram_tensor("bq", [NH * HD], F32, kind="ExternalInput")
    bk = nc.dram_tensor("bk", [HD], F32, kind="ExternalInput")
    bv = nc.dram_tensor("bv", [HD], F32, kind="ExternalInput")
    mask = nc.dram_tensor("mask", [P, NT, TW], FP, kind="ExternalInput")
    out = nc.dram_tensor("out", [T, C], FP, kind="ExternalOutput")

    xt_r = xt.ap().rearrange("(ko p) t -> p ko t", p=P)       # [128,16,2048]
    wq_r = wq.ap().rearrange("(ko p) m -> p ko m", p=P)       # [128,16,512]
    wk_r = wk.ap().rearrange("(ko p) m -> p ko m", p=P)       # [128,16,128]
    wv_r = wv.ap().rearrange("(ko p) m -> p ko m", p=P)
    wp_r = wp.ap().rearrange("(h p) o -> p h o", p=P)         # [128,4,2048]
    bq_r = bq.ap().rearrange("(h p) -> p h", p=P)             # [128,4]
    out_r = out.ap().rearrange("(io p) o -> p io o", p=P)     # [128,16,2048]

    with tile.TileContext(nc) as tc, ExitStack() as ctx:
        consts = ctx.enter_context(tc.tile_pool(name="consts", bufs=1))
        xpool = ctx.enter_context(tc.tile_pool(name="xpool", bufs=2))
        espool = ctx.enter_context(tc.tile_pool(name="espool", bufs=10))
        mpool = ctx.enter_context(tc.tile_pool(name="mpool", bufs=2))
        opool = ctx.enter_context(tc.tile_pool(name="opool", bufs=2))
        # PSUM (8 banks): ps_s 2x[128,2,512] (4) for S pairs + q/k proj,
        # ps_y 1x[128,512] (1), ps_d 1x[128,512] (1),
        # ps_o 2x[128,512] (2) for v-proj and out-proj quarters.
        ps_s = ctx.enter_context(tc.tile_pool(name="ps_s", bufs=2, space="PSUM"))
        ps_y = ctx.enter_context(tc.tile_pool(name="ps_y", bufs=1, space="PSUM"))
        ps_d = ctx.enter_context(tc.tile_pool(name="ps_d", bufs=1, space="PSUM"))
        ps_o = ctx.enter_context(tc.tile_pool(name="ps_o", bufs=2, space="PSUM"))

        # Weights needed first, loaded in k-chunks interleaved with the first
        # x tile so the first q matmul can start as early as possible.
        # Chunks stay >=0.125MB — smaller transfers halve effective HBM
        # bandwidth (per-transfer setup cost).
        KC = 4  # k-chunks per load
        wq_sb = consts.tile([P, KT, NH * HD], FP)
        wk_sb = consts.tile([P, KT, HD], FP)
        wv_sb = consts.tile([P, KT, HD], FP)
        xtile0 = xpool.tile([P, KT, TW], FP, tag="xt", name="xtile0")
        # Single k=0 slices first: the first q matmul needs only these 0.26MB.
        nc.sync.dma_start(out=xtile0[:, 0], in_=xt_r[:, 0, 0:TW])
        nc.sync.dma_start(out=wq_sb[:, 0], in_=wq_r[:, 0])
        for c4 in range(KC):
            ks = slice(max(c4 * (KT // KC), 1), (c4 + 1) * (KT // KC))
            nc.sync.dma_start(out=xtile0[:, ks], in_=xt_r[:, ks, 0:TW])
            nc.sync.dma_start(out=wq_sb[:, ks], in_=wq_r[:, ks])
        # wk is consumed after the q k-loop, wv by the v-projection at the
        # end of the n=0 loop — both well after the early x/wq slices.
        nc.sync.dma_start(out=wk_sb, in_=wk_r)
        for c4 in range(2):
            ks = slice(c4 * 8, (c4 + 1) * 8)
            nc.sync.dma_start(out=wv_sb[:, ks], in_=wv_r[:, ks])
        bq_sb = consts.tile([P, NH], F32)
        nc.sync.dma_start(out=bq_sb, in_=bq_r)
        bk_sb = consts.tile([P, 1], F32)
        nc.sync.dma_start(out=bk_sb, in_=bk.ap().rearrange("(h p) -> p h", p=P))
        # bv broadcast across partitions (DRAM source allows partition step 0).
        bv_bc = consts.tile([P, HD], F32)
        bv_ap = bass.AP(tensor=bv.ap().tensor, offset=0, ap=[[0, P], [1, HD]])
        nc.sync.dma_start(out=bv_bc, in_=bv_ap)
        ones_sb = consts.tile([P, P], FP)
        nc.vector.memset(ones_sb, 1.0)
        dummy_sb = consts.tile([P, TW], FP)
        nc.vector.memset(dummy_sb, 0.0)

        # PE warm-up: HAM un-throttles (1.2 -> 2.4 GHz) after ~3.4us of
        # sustained matmul activity.  Run throwaway matmuls while the first
        # input DMAs land so the real matmuls start at full clock.
        for w in range(12):
            ps_warm = ps_o.tile([P, TW], F32, tag="pso", name=f"ps_warm{w}")
            nc.tensor.matmul(
                ps_warm,
                lhsT=ones_sb,
                rhs=dummy_sb,
                start=True,
                stop=True,
            )

        # Persistent activations.
        qT = consts.tile([P, NH, T], FP)       # [d, h, i]
        kT = consts.tile([P, T], FP)           # [d, j]
        v_sb = consts.tile([P, JTN, HD], FP)   # [j_in, j_tile, d]
        yT = consts.tile([P, NH, T], FP)       # [d, h, i]
        mask_sb = consts.tile([P, NT, TW], FP)

        # ---- Projections ----
        xtiles = []

        def vproj_block(n, js, xtile):
            psv = ps_o.tile([P, TW], F32, tag="pso", name=f"psv_{n}_{js}")
            for k in range(KT):
                nc.tensor.matmul(
                    psv[:, :HD],
                    lhsT=xtile[:, k, js * P:(js + 1) * P],
                    rhs=wv_sb[:, k, :],
                    start=(k == 0),
                    stop=(k == KT - 1),
                )
            jt = n * (TW // P) + js
            nc.vector.tensor_tensor(
                out=v_sb[:, jt, :],
                in0=psv[:, :HD],
                in1=bv_bc,
                op=mybir.AluOpType.add,
            )

        for n in range(NT):
            if n == 0:
                xtile = xtile0
            else:
                xtile = xpool.tile([P, KT, TW], FP, tag="xt", name=f"xtile{n}")
                for c4 in range(KC):
                    ks = slice(c4 * (KT // KC), (c4 + 1) * (KT // KC))
                    nc.sync.dma_start(
                        out=xtile[:, ks], in_=xt_r[:, ks, n * TW:(n + 1) * TW]
                    )
                if n == 2:
                    # The causal mask is first read by the it=0 attention,
                    # which overlaps the n=3 projections; load it before
                    # xtile3 so it can't arrive late.
                    nc.sync.dma_start(out=mask_sb, in_=mask.ap())
            psq01 = ps_s.tile([P, 2, TW], F32, tag="pss", name=f"psq01_{n}")
            psq23 = ps_s.tile([P, 2, TW], F32, tag="pss", name=f"psq23_{n}")

            def kproj():
                psk = ps_y.tile([P, TW], F32, tag="psy", name=f"psk_{n}")
                for k in range(KT):
                    nc.tensor.matmul(
                        psk, lhsT=wk_sb[:, k, :], rhs=xtile[:, k, :],
                        start=(k == 0), stop=(k == KT - 1),
                    )
                nc.vector.tensor_scalar(
                    out=kT[:, n * TW:(n + 1) * TW],
                    in0=psk,
                    scalar1=bk_sb,
                    scalar2=None,
                    op0=mybir.AluOpType.add,
                )

            if n == NT - 1:
                # Last tile: k-projection first, so the psy bank it occupies
                # is freed long before the it=0 attention needs it.
                kproj()
            for k in range(KT):
                st = k == 0
                sp = k == KT - 1
                for h in range(NH):
                    tgt = psq01 if h < 2 else psq23
                    nc.tensor.matmul(
                        tgt[:, h % 2, :],
                        lhsT=wq_sb[:, k, h * HD:(h + 1) * HD],
                        rhs=xtile[:, k, :],
                        start=st,
                        stop=sp,
                    )
            if n < NT - 1:
                # k-projection after the q k-loop: by then the whole xtile is
                # in SBUF, so a late wk DMA can't stall the early stream.
                kproj()
            for h in range(NH):
                tgt = psq01 if h < 2 else psq23
                nc.vector.tensor_scalar(
                    out=qT[:, h, n * TW:(n + 1) * TW],
                    in0=tgt[:, h % 2, :],
                    scalar1=bq_sb[:, h:h + 1],
                    scalar2=None,
                    op0=mybir.AluOpType.add,
                )
            # v-projection: DMA-independent by now (q/k streamed the whole
            # xtile); one single-bank accumulator per 128-token block.
            xtiles.append(xtile)
            for js in range(TW // P):
                vproj_block(n, js, xtile)

        # Weights for the later phases: load after projection work is queued.
        wp_sb = consts.tile([P, NH, C], FP)
        nc.sync.dma_start(out=wp_sb, in_=wp_r)

        # ---- Attention with interleaved output projection ----
        # Out-proj for i-tile it is emitted between the attention heads of
        # i-tile it+1 (its yT rows are complete by then), so the PE always
        # has exp-independent matmuls to chew on while ACT computes exps.
        osb_map = {}

        def out_proj_quarters(ic, qs, final=False):
            # Quarter-granular: each [128, 512] output slab rotates through a
            # single-bank PSUM tile (2 bufs), so quarter q+1's matmuls run
            # while quarter q's PSUM->SBUF copy drains.  Copies alternate
            # between ACT and DVE to balance the two engines; each quarter is
            # stored as soon as its copy lands.
            if ic not in osb_map:
                osb_map[ic] = opool.tile([P, C], FP, tag="osb", name=f"osb_{ic}")
            osb = osb_map[ic]
            for q in qs:
                pso = ps_o.tile([P, TW], F32, tag="pso", name=f"pso_{ic}_{q}")
                for h in range(NH):
                    nc.tensor.matmul(
                        pso,
                        lhsT=yT[:, h, ic * P:(ic + 1) * P],
                        rhs=wp_sb[:, h, q * TW:(q + 1) * TW],
                        start=(h == 0),
                        stop=(h == NH - 1),
                    )
                if final and q >= 2:
                    # Split the trailing copies across both engines and issue
                    # the stores from those engines' own DMA queues (sync's
                    # queue serializes at ~700ns/store) so the kernel's tail
                    # drains as fast as possible.
                    for piece in range(2):
                        psl = slice(piece * HW, (piece + 1) * HW)
                        csl = slice(q * TW + piece * HW, q * TW + (piece + 1) * HW)
                        if piece == 0:
                            nc.scalar.copy(out=osb[:, csl], in_=pso[:, psl])
                            nc.scalar.dma_start(
                                out=out_r[:, ic, csl], in_=osb[:, csl]
                            )
                        else:
                            nc.vector.tensor_copy(out=osb[:, csl], in_=pso[:, psl])
                            nc.sync.dma_start(out=out_r[:, ic, csl], in_=osb[:, csl])
                else:
                    csl = slice(q * TW, (q + 1) * TW)
                    ceng = nc.scalar.copy if q % 2 == 0 else nc.vector.tensor_copy
                    ceng(out=osb[:, csl], in_=pso)
                    nc.sync.dma_start(out=out_r[:, ic, csl], in_=osb[:, csl])

        for it in range(NT):
            isl = slice(it * TW, (it + 1) * TW)
            npair = 2 * (it + 1)
            for h in range(NH):
                psy = ps_y.tile([P, TW], F32, tag="psy", name=f"psy_{it}_{h}")
                # fp16 DVE row-accumulator; den is one ones-matmul on the
                # pair-summed accumulator.
                acc = mpool.tile([P, 2, TW], FP, tag="acc", name=f"acc_{it}_{h}")
                for pr in range(npair):
                    jt0 = 2 * pr
                    last = pr == npair - 1
                    penu = pr == npair - 2
                    # The diagonal pair only serves queries [HW:TW) of the
                    # i-tile; everything below is fully masked anyway.
                    fw = HW if last else TW
                    kdiag = jt0 - it * (TW // P)
                    pss = ps_s.tile(
                        [P, 2, TW], F32, tag="pss", name=f"pss_{it}_{h}_{pr}"
                    )
                    base = HW if last else 0
                    for u in range(2):
                        jt = jt0 + u
                        # Diagonal j-tiles only serve queries >= their own
                        # row block; skip the S columns below it.  The exp
                        # of the stale (bounded) PSUM there is zeroed by the
                        # causal mask before anything consumes it.
                        s0 = max(jt * P - it * TW - base, 0) if kdiag >= 0 else 0
                        nc.tensor.matmul(
                            pss[:, u, s0:fw],
                            lhsT=kT[:, jt * P:(jt + 1) * P],
                            rhs=qT[:, h, it * TW + base + s0:(it + 1) * TW],
                            start=True,
                            stop=True,
                        )
                    es = espool.tile([P, 2, TW], FP, tag="es")
                    if fw == TW:
                        # Both u-halves are contiguous (the pair tile spans
                        # two adjacent PSUM banks) — one flat 2D activation.
                        nc.scalar.activation(
                            out=es[:].rearrange("p a b -> p (a b)"),
                            in_=pss[:].rearrange("p a b -> p (a b)"),
                            func=mybir.ActivationFunctionType.Exp,
                            scale=SCALE,
                        )
                    else:
                        nc.scalar.activation(
                            out=es[:, :, :fw],
                            in_=pss[:, :, :fw],
                            func=mybir.ActivationFunctionType.Exp,
                            scale=SCALE,
                        )
                    if kdiag >= 0:
                        # The S trim already skipped everything above the
                        # diagonal except each j-tile's own 128-column block;
                        # only that block needs the 0/1 mask.  Shorter DVE
                        # ops also shorten the exp->AV critical chain.
                        for u in range(2):
                            s0u = max((jt0 + u) * P - it * TW - base, 0)
                            nc.vector.tensor_mul(
                                es[:, u, s0u:s0u + P],
                                es[:, u, s0u:s0u + P],
                                mask_sb[:, kdiag + u, base + s0u:base + s0u + P],
                            )
                    if pr == 0:
                        if kdiag >= 0:
                            # it=0 first pair: u=1's region below its own
                            # block was never computed — zero it in acc.
                            nc.vector.tensor_copy(out=acc[:, 0, :], in_=es[:, 0, :])
                            nc.vector.memset(acc[:, 1, 0:P], 0.0)
                            nc.vector.tensor_copy(
                                out=acc[:, 1, P:], in_=es[:, 1, P:]
                            )
                        else:
                            nc.vector.tensor_copy(
                                out=acc[:].rearrange("p a b -> p (a b)"),
                                in_=es[:].rearrange("p a b -> p (a b)"),
                            )
                    elif kdiag >= 0:
                        for u in range(2):
                            s0u = max((jt0 + u) * P - it * TW - base, 0)
                            nc.vector.tensor_tensor(
                                out=acc[:, u, base + s0u:],
                                in0=acc[:, u, base + s0u:],
                                in1=es[:, u, s0u:fw],
                                op=mybir.AluOpType.add,
                            )
                    else:
                        acc2 = acc[:].rearrange("p a b -> p (a b)")
                        nc.vector.tensor_tensor(
                            out=acc2,
                            in0=acc2,
                            in1=es[:].rearrange("p a b -> p (a b)"),
                            op=mybir.AluOpType.add,
                        )
                    # AV accumulation into psy, trimmed like S: diagonal
                    # j-tiles only contribute to queries >= their own row
                    # block (the skipped region's es is masked zeros anyway).
                    # `stop` is sim-only metadata; the ymul read is ordered
                    # by data dependency, so ragged groups are fine.
                    for u in range(2):
                        jt = jt0 + u
                        s0 = max(jt * P - it * TW - base, 0) if kdiag >= 0 else 0
                        nc.tensor.matmul(
                            psy[:, base + s0:],
                            lhsT=v_sb[:, jt, :],
                            rhs=es[:, u, s0:fw],
                            start=(jt == 0),
                            stop=(pr == npair - 1 and u == 1),
                            skip_group_check=True,
                        )
                accs = mpool.tile([P, TW], FP, tag="accs", name=f"accs_{it}_{h}")
                nc.vector.tensor_tensor(
                    out=accs, in0=acc[:, 0, :], in1=acc[:, 1, :],
                    op=mybir.AluOpType.add,
                )
                psd = ps_d.tile([P, TW], F32, tag="psd", name=f"psd_{it}_{h}")
                nc.tensor.matmul(psd, lhsT=ones_sb, rhs=accs, start=True, stop=True)
                rb = mpool.tile([P, TW], F32, tag="rb")
                if it == NT - 1 and h == NH - 1:
                    # The tail chunks gate on this head's yT; pipeline the
                    # recip+ymul in 128-column pieces so out-proj chunk 12's
                    # last matmul unblocks after the first piece.
                    for qq in range(4):
                        sl = slice(qq * P, (qq + 1) * P)
                        gsl = slice(it * TW + qq * P, it * TW + (qq + 1) * P)
                        nc.vector.reciprocal_approx_fast(
                            out=rb[:, sl], in_=psd[:, sl]
                        )
                        nc.vector.tensor_mul(yT[:, h, gsl], psy[:, sl], rb[:, sl])
                else:
                    nc.vector.reciprocal_approx_fast(out=rb, in_=psd)
                    nc.vector.tensor_mul(yT[:, h, isl], psy, rb)
                if it > 0:
                    out_proj_quarters((it - 1) * (TW // P) + h, (0, 1, 2, 3))
        # Tail chunks for the last i-tile's rows.
        for h in range(NH):
            out_proj_quarters(
                (NT - 1) * (TW // P) + h, (0, 1, 2, 3), final=(h == NH - 1)
            )

    nc.compile()
    return nc


def _causal_mask_tiles():
    j = np.arange(P)[:, None, None]
    k = np.arange(NT)[None, :, None]
    i = np.arange(TW)[None, None, :]
    return (j + k * P <= i).astype(np.float16)


def kernel(x, Wkv, bkv, Wq, bq, Wp, bp):
    global LAST_RESULT
    x = np.asarray(x, np.float32)
    Wkv = np.asarray(Wkv, np.float32)
    bkv = np.asarray(bkv, np.float32)
    Wq = np.asarray(Wq, np.float32)
    bq = np.asarray(bq, np.float32)
    Wp = np.asarray(Wp, np.float32)
    bp = np.asarray(bp, np.float32)

    if "nc" not in _CACHE:
        _CACHE["nc"] = _build_bass()
    nc = _CACHE["nc"]

    mask = _causal_mask_tiles()
    CG = C // G  # 512 columns per kv head in the k/v halves of Wkv

    in_maps = []
    for b in range(B):
        xt = x[b].T.astype(np.float16)
        for g in range(HKV):
            heads = [g + HKV * u for u in range(NH)]  # h % HKV == g
            wq_g = np.concatenate(
                [Wq[:, h * HD:(h + 1) * HD] for h in heads], axis=1
            ).astype(np.float16)
            bq_g = np.concatenate([bq[h * HD:(h + 1) * HD] for h in heads])
            wp_g = np.ascontiguousarray(
                np.concatenate([Wp[h * HD:(h + 1) * HD, :] for h in heads], axis=0)
            ).astype(np.float16)
            wk_g = np.ascontiguousarray(Wkv[:, g * HD:(g + 1) * HD]).astype(np.float16)
            wv_g = np.ascontiguousarray(
                Wkv[:, CG + g * HD:CG + (g + 1) * HD]
            ).astype(np.float16)
            bk_g = np.ascontiguousarray(bkv[g * HD:(g + 1) * HD])
            bv_g = np.ascontiguousarray(bkv[CG + g * HD:CG + (g + 1) * HD])
            in_maps.append(
                {
                    "xt": xt,
                    "wq": wq_g,
                    "wk": wk_g,
                    "wv": wv_g,
                    "wp": wp_g,
                    "bq": np.ascontiguousarray(bq_g, np.float32),
                    "bk": np.ascontiguousarray(bk_g, np.float32),
                    "bv": np.ascontiguousarray(bv_g, np.float32),
                    "mask": mask,
                }
            )

    res = bass_utils.run_bass_kernel_spmd(nc, in_maps, core_ids=list(range(B * HKV)))
    LAST_RESULT = res

    out = np.zeros((B, T, C), np.float32)
    for b in range(B):
        acc = np.zeros((T, C), np.float32)
        for g in range(HKV):
            acc += res.results[b * HKV + g]["out"]
        out[b] = acc + bp[None, :]
    return out
